# revision 1
# baseline (speedup 1.0000x reference)
"""MixProp GNN message-passing kernel for 8 TRN2 NeuronCores.

Reference computation (per batch element b):
    A_n = row_normalize(A + I)
    H_0 = X;  H_k = beta*X + (1-beta) * A_n @_nodes H_{k-1}   (k=1..3)
    out = W @_channels concat([H_0..H_3]) + bias

Kernel strategy:
  - Data-parallel over batch: B=8 batch elements -> 8 cores, no collectives.
  - Host precomputes G_k = polynomial in A_n s.t. H_k = G_k @ X (node-mixing
    and channel-mixing commute, and the hop recurrence is affine in X).
    This removes the sequential hop chain and the beta*X elementwise adds.
  - On device, for each seq position l (all ops are pointwise in l):
      * 4 column-packed matmuls (tile_position=(0,32j)) per 128-node block
        of the contraction build a PSUM tile H0[(src,ch), v] where src =
        (X, G1X, G2X, G3X): lhsT = X[:, l-slice] (stationary, m=32,
        contiguous), rhs = I / G_k^T (moving). The four column groups run
        concurrently on the PE's 32-column strips, so the X "transpose"
        (identity matmul, group 0) adds no wall time.
      * one k=128 conv matmul pair vs W^T produces out[(vh,o), v-half],
        placed in psum partitions 0:64 / 64:128 so the output store runs
        at full 128-partition DMA width.
  - Constants are pre-cast to bf16 on host; X is loaded f32 (HWDGE) and a
    DVE copy casts + reorders its free dim (c,l)->(l,c) so lhsT slices
    are contiguous (strided LDWEIGHTS is ~6x slower).
  - PSUM->SBUF evacuation and bias-add on DVE.
"""

import sys

sys.path.insert(0, "/opt/trn_rl_repo")

import numpy as np

import concourse.bass as bass
import concourse.bacc as bacc
import concourse.mybir as mybir
from concourse import tile
from concourse import bass_utils

GDEP = 3
BETA = 0.05
C_IN = 32
C_OUT = 64
N = 512
B = 8
L = 256
NB = N // 128  # node blocks of 128

F32 = mybir.dt.float32


class CFG:
    def __init__(self, L=L, Lc=32, mm_dt=mybir.dt.bfloat16):
        assert L % Lc == 0
        self.L = L
        self.Lc = Lc
        self.mm_dt = mm_dt


def body(nc, tc, outs, ins, cfg: CFG):
    """Emit the per-core program. ins/outs are dicts of DRAM APs."""
    X_d = ins["x"]          # [C_IN, N, L] f32
    G_d = ins["gt"]         # [GDEP, N, N] f32  G_k^T
    I_d = ins["ident"]      # [128, 128]   f32
    W_d = ins["wt"]         # [128, C_OUT] f32  W^T
    b_d = ins["bias2"]      # [128, 1]     f32  bias duplicated for (vh, o)
    out_d = outs["out"]     # [C_OUT, N, L] f32

    Lc = cfg.Lc
    mm_dt = cfg.mm_dt
    n_chunks = cfg.L // Lc

    with (
        tc.tile_pool(name="const", bufs=1) as cpool,
        tc.tile_pool(name="xraw", bufs=2) as xraw_pool,
        tc.tile_pool(name="xsb", bufs=2) as xsb_pool,
        tc.tile_pool(name="h0sb", bufs=4) as h0sb_pool,
        tc.tile_pool(name="outsb", bufs=2) as out_pool,
        tc.tile_pool(name="h0ps", bufs=2, space="PSUM") as h0ps_pool,
        tc.tile_pool(name="cvps", bufs=2, space="PSUM") as cvps_pool,
    ):
        # ---- constants (pre-cast to bf16 on host, plain HWDGE loads) ----
        g_t = []  # g_t[k][wb]: [128, N] moving operand for hop k
        for k in range(GDEP):
            row = []
            for wb in range(NB):
                t = cpool.tile([128, N], mm_dt, name=f"g{k}_{wb}")
                nc.sync.dma_start(t[:], G_d[k, wb * 128:(wb + 1) * 128, :])
                row.append(t)
            g_t.append(row)
        i_t = []
        for wb in range(NB):
            t = cpool.tile([128, N], mm_dt, name=f"i_{wb}")
            nc.sync.dma_start(t[:], I_d[wb * 128:(wb + 1) * 128, :])
            i_t.append(t)
        w_t = cpool.tile([128, C_OUT], mm_dt, name="w_t")
        nc.sync.dma_start(w_t[:], W_d[:])
        b_t = cpool.tile([128, 1], F32, name="b_t")
        nc.sync.dma_start(b_t[:], b_d[:])

        for ch in range(n_chunks):
            lsl = slice(ch * Lc, (ch + 1) * Lc)

            # ---- X load (cast to bf16 in DMA), layout [w, (c, l)],
            # then DVE reorder to [w, (l, c)] so lhsT slices are
            # contiguous (strided LDWEIGHTS costs ~6x). ----
            xsb_tiles = []
            for wb in range(NB):
                xraw = xraw_pool.tile(
                    [128, C_IN * Lc], F32, name="xraw", tag=f"xraw{wb}"
                )
                src = X_d[:, wb * 128:(wb + 1) * 128, lsl].rearrange(
                    "c w l -> w c l"
                )
                nc.sync.dma_start(
                    xraw.rearrange("w (c l) -> w c l", l=Lc), src
                )
                xsb = xsb_pool.tile(
                    [128, Lc * C_IN], mm_dt, name="xsb", tag=f"xsb{wb}"
                )
                nc.vector.tensor_copy(
                    out=xsb.rearrange("w (l c) -> w c l", c=C_IN),
                    in_=xraw.rearrange("w (c l) -> w c l", l=Lc),
                )
                xsb_tiles.append(xsb)

            out_sb = out_pool.tile([128, 256 * Lc], F32, name="out_sb")
            out_v = out_sb.rearrange("p (v l) -> p v l", l=Lc)

            # ---- per-seq-position pipeline ----
            prev = None  # deferred conv so PE never waits on the DVE evac
            for l0 in range(Lc):
                h0p = h0ps_pool.tile([128, N], F32, name="h0p")
                for wb in range(NB):
                    st = wb == 0
                    sp = wb == NB - 1
                    xl = xsb_tiles[wb][:, l0 * C_IN:(l0 + 1) * C_IN]
                    # X-transpose: identity matmul in column group 0.
                    # Full n=512 so it pipelines at the same stream rate
                    # as the hop matmuls (short-n MMs pay the dispatch
                    # floor instead).
                    nc.tensor.matmul(
                        h0p[0:32, :], lhsT=xl, rhs=i_t[wb][:],
                        start=st, stop=sp,
                        tile_position=(0, 0), skip_group_check=True,
                    )
                    for k in range(GDEP):
                        j = k + 1
                        nc.tensor.matmul(
                            h0p[32 * j:32 * (j + 1), :], lhsT=xl,
                            rhs=g_t[k][wb][:],
                            start=st, stop=sp, tile_position=(0, 32 * j),
                            skip_group_check=True,
                        )
                h0s = h0sb_pool.tile([128, N], mm_dt, name="h0s")
                nc.vector.tensor_copy(out=h0s[:], in_=h0p[:])

                if prev is not None:
                    _emit_conv(nc, cvps_pool, w_t, b_t, out_v, *prev)
                prev = (h0s, l0)
            _emit_conv(nc, cvps_pool, w_t, b_t, out_v, *prev)

            # ---- store chunk ----
            for vh in range(2):
                nc.sync.dma_start(
                    out_d[:, vh * 256:(vh + 1) * 256, lsl],
                    out_sb[vh * 64:(vh + 1) * 64, :].rearrange(
                        "o (v l) -> o v l", l=Lc
                    ),
                )


def _emit_conv(nc, cvps_pool, w_t, b_t, out_v, h0s, l0):
    cvp = cvps_pool.tile([128, 256], F32, name="cvp")
    nc.tensor.matmul(
        cvp[0:64, :], lhsT=w_t[:], rhs=h0s[:, 0:256],
        start=True, stop=True, tile_position=(0, 0),
        skip_group_check=True,
    )
    nc.tensor.matmul(
        cvp[64:128, :], lhsT=w_t[:], rhs=h0s[:, 256:512],
        start=True, stop=True, tile_position=(0, 64),
        skip_group_check=True,
    )
    nc.vector.tensor_scalar_add(
        out=out_v[:, :, l0], in0=cvp[:], scalar1=b_t[:, 0:1]
    )


def build_nc(cfg: CFG):
    nc = bacc.Bacc("TRN2", target_bir_lowering=False, debug=False)
    ins = {
        "x": nc.dram_tensor("x", [C_IN, N, cfg.L], F32,
                            kind="ExternalInput").ap(),
        "gt": nc.dram_tensor("gt", [GDEP, N, N], cfg.mm_dt,
                             kind="ExternalInput").ap(),
        "ident": nc.dram_tensor("ident", [N, N], cfg.mm_dt,
                                kind="ExternalInput").ap(),
        "wt": nc.dram_tensor("wt", [128, C_OUT], cfg.mm_dt,
                             kind="ExternalInput").ap(),
        "bias2": nc.dram_tensor("bias2", [128, 1], F32,
                                kind="ExternalInput").ap(),
    }
    outs = {
        "out": nc.dram_tensor("out", [C_OUT, N, cfg.L], F32,
                              kind="ExternalOutput").ap(),
    }
    with tile.TileContext(nc) as tc:
        body(nc, tc, outs, ins, cfg)
    nc.compile()
    return nc


def make_host_inputs(A, W, b):
    """Precompute the replicated operands: G_k^T, I, W^T, bias2."""
    A = np.asarray(A, np.float64)
    n = A.shape[0]
    An = A + np.eye(n)
    An = An / An.sum(axis=1, keepdims=True)
    As = (1.0 - BETA) * An
    eye = np.eye(n)
    G = []
    gk = eye
    for _ in range(GDEP):
        gk = As @ gk + BETA * eye
        G.append(gk)
    import ml_dtypes
    bf16 = ml_dtypes.bfloat16
    GT = np.stack([g.T for g in G]).astype(bf16)  # [GDEP, N, N]
    ident = np.eye(n, dtype=bf16)
    WT = np.ascontiguousarray(np.asarray(W, np.float64).T.astype(bf16))
    b = np.asarray(b, np.float32)
    b2 = np.ascontiguousarray(np.concatenate([b, b]).reshape(128, 1))
    return GT, ident, WT, b2


_NC_CACHE = {}


def run_on_hw(X, A, W, b, cfg=None, trace=False, **spmd_kwargs):
    X = np.ascontiguousarray(np.asarray(X, np.float32))
    GT, ident, WT, b2 = make_host_inputs(A, W, b)
    if cfg is None:
        cfg = CFG()
    key = (cfg.L, cfg.Lc, cfg.mm_dt)
    if key not in _NC_CACHE:
        _NC_CACHE[key] = build_nc(cfg)
    nc = _NC_CACHE[key]
    in_maps = [
        {"x": X[i], "gt": GT, "ident": ident, "wt": WT, "bias2": b2}
        for i in range(B)
    ]
    res = bass_utils.run_bass_kernel_spmd(
        nc, in_maps, core_ids=list(range(B)), trace=trace, **spmd_kwargs
    )
    out = np.stack([res.results[i]["out"] for i in range(B)])
    return out, res


def kernel(X, A, W, b):
    return run_on_hw(X, A, W, b)[0]


if __name__ == "__main__":
    rng = np.random.default_rng(0)
    X = rng.standard_normal((B, C_IN, N, L), dtype=np.float32)
    A = rng.random((N, N), dtype=np.float32)
    W = rng.standard_normal((C_OUT, (GDEP + 1) * C_IN), dtype=np.float32) * 0.1
    b = rng.random(C_OUT, dtype=np.float32)
    out = kernel(X, A, W, b)
    print("out", out.shape, out.dtype, float(np.abs(out).mean()))



# revision 2
# speedup vs baseline: 1.3760x; 1.3760x over previous
"""MixProp GNN message-passing kernel for 8 TRN2 NeuronCores.

Reference computation (per batch element b):
    A_n = row_normalize(A + I)
    H_0 = X;  H_k = beta*X + (1-beta) * A_n @_nodes H_{k-1}   (k=1..3)
    out = W @_channels concat([H_0..H_3]) + bias

Kernel strategy (v2):
  - Data-parallel over batch: B=8 batch elements -> 8 cores, no collectives.
  - Host precomputes G_k s.t. H_k = G_k @ X (hop recurrence is affine in X),
    pre-casts all matmul operands to bf16, and pre-transposes X into the
    exact lhsT layout [wb, w, l, c] the PE wants, so the device does zero
    layout work on X (the v1 kernel burned DVE time + a 131k-descriptor DMA
    storm transposing X on the fly).
  - Per seq position l: 4 column-packed matmuls per 128-node block build
    PSUM H0[(src,ch), v] (src = X via identity / G1 / G2 / G3); the four
    column groups run concurrently on the PE's 32-column strips.
  - PSUM->SBUF evacuation alternates between DVE and the (otherwise idle)
    Scalar engine per l; the conv output evac carries the bias add
    (tensor_scalar_add on DVE / activation-Identity-with-bias on Scalar).
  - Output staged bf16 in SBUF as [(vh,o), (l, v)] per 32-l chunk and
    stored to a chunk-major DRAM layout (16 KB contiguous per partition,
    ~128 descriptors per store vs 16k scattered 128 B ones in v1). The
    host reassembles [64, 512, 256] f32 (bf16 rounding of the output is
    ~0.4% rel, well inside the 2e-2 gate).
  - ~32 warmup matmuls on the identity tile run during the initial X/const
    DMA fill so the PE's HAM clock-gate is at 8/8 when real work starts.
"""

import sys

sys.path.insert(0, "/opt/trn_rl_repo")

import numpy as np

import concourse.bass as bass
import concourse.bacc as bacc
import concourse.mybir as mybir
from concourse import tile
from concourse import bass_utils

GDEP = 3
BETA = 0.05
C_IN = 32
C_OUT = 64
N = 512
B = 8
L = 256
NB = N // 128  # node blocks of 128

F32 = mybir.dt.float32
BF16 = mybir.dt.bfloat16
IDENT = mybir.ActivationFunctionType.Identity


class CFG:
    def __init__(self, L=L, Lc=32, n_warm=32):
        assert L % Lc == 0
        self.L = L
        self.Lc = Lc
        self.n_warm = n_warm


def body(nc, tc, outs, ins, cfg: CFG):
    """Emit the per-core program. ins/outs are dicts of DRAM APs."""
    X_d = ins["xw"]         # [NB, 128, L, C_IN] bf16  pre-transposed lhsT
    G_d = ins["gt"]         # [GDEP, N, N] bf16  G_k^T
    I_d = ins["ident"]      # [N, N]       bf16
    W_d = ins["wt"]         # [128, C_OUT] bf16  W^T
    b_d = ins["bias2"]      # [128, 1]     f32   bias duplicated for (vh, o)
    out_d = outs["out"]     # [n_chunks, 128, Lc, 256] bf16 chunk-major

    Lc = cfg.Lc
    n_chunks = cfg.L // Lc

    with (
        tc.tile_pool(name="const", bufs=1) as cpool,
        tc.tile_pool(name="h0sb", bufs=4) as h0sb_pool,
        tc.tile_pool(name="outsb", bufs=2) as out_pool,
        tc.tile_pool(name="h0ps", bufs=3, space="PSUM") as h0ps_pool,
        tc.tile_pool(name="cvps", bufs=2, space="PSUM") as cvps_pool,
        tc.tile_pool(name="wmps", bufs=1, space="PSUM") as wm_pool,
    ):
        # ---- constants (pre-cast to bf16 on host, plain HWDGE loads) ----
        i_t = []
        for wb in range(NB):
            t = cpool.tile([128, N], BF16, name=f"i_{wb}")
            nc.sync.dma_start(t[:], I_d[wb * 128:(wb + 1) * 128, :])
            i_t.append(t)
        g_t = []  # g_t[k][wb]: [128, N] moving operand for hop k
        for k in range(GDEP):
            row = []
            for wb in range(NB):
                t = cpool.tile([128, N], BF16, name=f"g{k}_{wb}")
                nc.sync.dma_start(t[:], G_d[k, wb * 128:(wb + 1) * 128, :])
                row.append(t)
            g_t.append(row)
        w_t = cpool.tile([128, C_OUT], BF16, name="w_t")
        nc.sync.dma_start(w_t[:], W_d[:])
        b_t = cpool.tile([128, 1], F32, name="b_t")
        nc.sync.dma_start(b_t[:], b_d[:])

        # ---- X: one contiguous 2 MB DMA per node block, SBUF-resident ----
        xw = []
        for wb in range(NB):
            t = cpool.tile([128, cfg.L * C_IN], BF16, name=f"xw_{wb}")
            nc.sync.dma_start(
                t.rearrange("w (l c) -> w l c", c=C_IN), X_d[wb]
            )
            xw.append(t)

        # ---- HAM warmup: junk matmuls on i_t[0] while X loads ----
        wm = wm_pool.tile([128, N], F32, name="wm")
        for _ in range(cfg.n_warm):
            nc.tensor.matmul(
                wm[:], lhsT=i_t[0][:, 0:128], rhs=i_t[0][:],
                start=True, stop=True,
            )

        def emit_conv(h0s, l):
            """Channel-mix conv for position l + bias, into out_sb."""
            cvp = cvps_pool.tile([128, 256], F32, name="cvp")
            nc.tensor.matmul(
                cvp[0:64, :], lhsT=w_t[:], rhs=h0s[:, 0:256],
                start=True, stop=True, tile_position=(0, 0),
                skip_group_check=True,
            )
            nc.tensor.matmul(
                cvp[64:128, :], lhsT=w_t[:], rhs=h0s[:, 256:512],
                start=True, stop=True, tile_position=(0, 64),
                skip_group_check=True,
            )
            dst = out_sb[:, (l % Lc) * 256:(l % Lc + 1) * 256]
            if l % 2 == 0:
                nc.scalar.add(dst, cvp[:], b_t[:, 0:1])
            else:
                nc.vector.tensor_scalar_add(
                    out=dst, in0=cvp[:], scalar1=b_t[:, 0:1]
                )

        out_sb = out_pool.tile([128, 256 * Lc], BF16, name="out_sb")
        prev = None  # deferred conv so PE never waits on the evac
        for l in range(cfg.L):
            h0p = h0ps_pool.tile([128, N], F32, name="h0p")
            for wb in range(NB):
                st = wb == 0
                sp = wb == NB - 1
                xl = xw[wb][:, l * C_IN:(l + 1) * C_IN]
                # X-transpose: identity matmul in column group 0 rides
                # concurrently with the three hop matmuls.
                nc.tensor.matmul(
                    h0p[0:32, :], lhsT=xl, rhs=i_t[wb][:],
                    start=st, stop=sp,
                    tile_position=(0, 0), skip_group_check=True,
                )
                for k in range(GDEP):
                    j = k + 1
                    nc.tensor.matmul(
                        h0p[32 * j:32 * (j + 1), :], lhsT=xl,
                        rhs=g_t[k][wb][:],
                        start=st, stop=sp, tile_position=(0, 32 * j),
                        skip_group_check=True,
                    )
            h0s = h0sb_pool.tile([128, N], BF16, name="h0s")
            if l % 2 == 0:
                nc.vector.tensor_copy(out=h0s[:], in_=h0p[:])
            else:
                nc.scalar.copy(h0s[:], h0p[:])

            if prev is not None:
                emit_conv(*prev)
                lp = prev[1]
                if lp % Lc == Lc - 1:  # chunk complete -> store it
                    ch = lp // Lc
                    nc.sync.dma_start(
                        out_d[ch],
                        out_sb.rearrange("p (l v) -> p l v", v=256),
                    )
                    if ch + 1 < n_chunks:
                        out_sb = out_pool.tile(
                            [128, 256 * Lc], BF16, name="out_sb"
                        )
            prev = (h0s, l)
        emit_conv(*prev)
        nc.sync.dma_start(
            out_d[n_chunks - 1],
            out_sb.rearrange("p (l v) -> p l v", v=256),
        )


def build_nc(cfg: CFG):
    nc = bacc.Bacc("TRN2", target_bir_lowering=False, debug=False)
    n_chunks = cfg.L // cfg.Lc
    ins = {
        "xw": nc.dram_tensor("xw", [NB, 128, cfg.L, C_IN], BF16,
                             kind="ExternalInput").ap(),
        "gt": nc.dram_tensor("gt", [GDEP, N, N], BF16,
                             kind="ExternalInput").ap(),
        "ident": nc.dram_tensor("ident", [N, N], BF16,
                                kind="ExternalInput").ap(),
        "wt": nc.dram_tensor("wt", [128, C_OUT], BF16,
                             kind="ExternalInput").ap(),
        "bias2": nc.dram_tensor("bias2", [128, 1], F32,
                                kind="ExternalInput").ap(),
    }
    outs = {
        "out": nc.dram_tensor("out", [n_chunks, 128, cfg.Lc, 256], BF16,
                              kind="ExternalOutput").ap(),
    }
    with tile.TileContext(nc) as tc:
        body(nc, tc, outs, ins, cfg)
    nc.compile()
    return nc


def make_host_inputs(X, A, W, b):
    """Precompute all device operands on the host."""
    import ml_dtypes
    bf16 = ml_dtypes.bfloat16

    A = np.asarray(A, np.float64)
    n = A.shape[0]
    An = A + np.eye(n)
    An = An / An.sum(axis=1, keepdims=True)
    As = (1.0 - BETA) * An
    eye = np.eye(n)
    G = []
    gk = eye
    for _ in range(GDEP):
        gk = As @ gk + BETA * eye
        G.append(gk)
    GT = np.stack([g.T for g in G]).astype(bf16)  # [GDEP, N, N]
    ident = np.eye(n, dtype=bf16)
    WT = np.ascontiguousarray(np.asarray(W, np.float64).T.astype(bf16))
    b = np.asarray(b, np.float32)
    b2 = np.ascontiguousarray(np.concatenate([b, b]).reshape(128, 1))

    # X [B, C_IN, N, L] f32 -> per core [NB, 128, L, C_IN] bf16 (lhsT layout)
    X = np.asarray(X)
    XW = np.ascontiguousarray(X.transpose(0, 2, 3, 1)).astype(bf16)
    XW = XW.reshape(B, NB, 128, L, C_IN)
    return XW, GT, ident, WT, b2


_NC_CACHE = {}


def run_on_hw(X, A, W, b, cfg=None, trace=False, **spmd_kwargs):
    XW, GT, ident, WT, b2 = make_host_inputs(X, A, W, b)
    if cfg is None:
        cfg = CFG()
    key = (cfg.L, cfg.Lc, cfg.n_warm)
    if key not in _NC_CACHE:
        _NC_CACHE[key] = build_nc(cfg)
    nc = _NC_CACHE[key]
    in_maps = [
        {"xw": XW[i], "gt": GT, "ident": ident, "wt": WT, "bias2": b2}
        for i in range(B)
    ]
    res = bass_utils.run_bass_kernel_spmd(
        nc, in_maps, core_ids=list(range(B)), trace=trace, **spmd_kwargs
    )
    # out_dev [n_chunks, 128=(vh,o), Lc, 256=v] bf16
    #   -> out [C_OUT, N, L] f32  via (o, vh, v, ch, l)
    n_chunks = cfg.L // cfg.Lc
    outs = []
    for i in range(B):
        o = np.asarray(res.results[i]["out"])
        o = o.reshape(n_chunks, 2, C_OUT, cfg.Lc, 256)
        o = o.transpose(2, 1, 4, 0, 3).reshape(C_OUT, N, cfg.L)
        outs.append(o.astype(np.float32))
    return np.stack(outs), res


def kernel(X, A, W, b):
    return run_on_hw(X, A, W, b)[0]


if __name__ == "__main__":
    rng = np.random.default_rng(0)
    X = rng.standard_normal((B, C_IN, N, L), dtype=np.float32)
    A = rng.random((N, N), dtype=np.float32)
    W = rng.standard_normal((C_OUT, (GDEP + 1) * C_IN), dtype=np.float32) * 0.1
    b = rng.random(C_OUT, dtype=np.float32)
    out = kernel(X, A, W, b)
    print("out", out.shape, out.dtype, float(np.abs(out).mean()))


# revision 3
# speedup vs baseline: 1.6047x; 1.1662x over previous
"""MixProp GNN message-passing kernel for 8 TRN2 NeuronCores.

Reference computation (per batch element b):
    A_n = row_normalize(A + I)
    H_0 = X;  H_k = beta*X + (1-beta) * A_n @_nodes H_{k-1}   (k=1..3)
    out = W @_channels concat([H_0..H_3]) + bias

Kernel strategy (v3):
  - Data-parallel over batch: B=8 batch elements -> 8 cores, no collectives.
  - Host precomputes G_k s.t. H_k = G_k @ X, pre-casts operands to bf16 and
    pre-transposes X into the lhsT layout [wb, w, l, c]: the device does no
    layout work on X.
  - Per seq position l: per 128-node block, 4 column-packed matmuls (X via a
    single 128x128 identity with per-block start/stop, and G1..G3) build
    PSUM H0[(src,ch), v]; the column groups stream concurrently so each l
    costs ~4x512 moving columns (~850 ns warm).
  - Seq positions are paired for the channel conv: both l's H0 go into one
    [128, 1024] SBUF tile (v-half-major), the conv is 2 matmuls of n=512
    into one PSUM bank, evacuated (+bias) in a single op.
  - PSUM->SBUF evacuation alternates DVE / Scalar engine per l.
  - Output staged bf16 as [(vh,o), (l, v)] per 32-l chunk, stored to a
    chunk-major DRAM layout (16 KB contiguous per partition); the host
    reassembles [64, 512, 256] f32 (bf16 output rounding ~0.4% rel, well
    inside the 2e-2 gate).
  - DMA order: first G tile lands in ~1 us and feeds ~48 HAM-warmup
    matmuls; X arrives in 64-l slices, first-needed first, so real compute
    starts ~12 us in instead of waiting for the full 10 MB fill.
"""

import sys

sys.path.insert(0, "/opt/trn_rl_repo")

import numpy as np

import concourse.bass as bass
import concourse.bacc as bacc
import concourse.mybir as mybir
from concourse import tile
from concourse import bass_utils

GDEP = 3
BETA = 0.05
C_IN = 32
C_OUT = 64
N = 512
B = 8
L = 256
NB = N // 128  # node blocks of 128

F32 = mybir.dt.float32
BF16 = mybir.dt.bfloat16


class CFG:
    def __init__(self, L=L, Lc=32, Lq=64, n_warm=48):
        assert L % Lc == 0 and L % Lq == 0
        self.L = L
        self.Lc = Lc      # output store chunk
        self.Lq = Lq      # X load slice
        self.n_warm = n_warm


def body(nc, tc, outs, ins, cfg: CFG):
    """Emit the per-core program. ins/outs are dicts of DRAM APs."""
    X_d = ins["xw"]         # [NB, 128, L, C_IN] bf16  pre-transposed lhsT
    G_d = ins["gt"]         # [GDEP, N, N] bf16  G_k^T
    I_d = ins["ident"]      # [128, 128]   bf16
    W_d = ins["wt"]         # [128, C_OUT] bf16  W^T
    b_d = ins["bias2"]      # [128, 1]     f32   bias duplicated for (vh, o)
    out_d = outs["out"]     # [n_chunks, 128, Lc, 256] bf16 chunk-major

    Lc, Lq = cfg.Lc, cfg.Lq
    n_chunks = cfg.L // Lc
    n_xq = cfg.L // Lq

    with (
        tc.tile_pool(name="const", bufs=1) as cpool,
        tc.tile_pool(name="h0sb", bufs=3) as h0sb_pool,
        tc.tile_pool(name="outsb", bufs=2) as out_pool,
        tc.tile_pool(name="h0ps", bufs=3, space="PSUM") as h0ps_pool,
        tc.tile_pool(name="cvps", bufs=2, space="PSUM") as cvps_pool,
        tc.tile_pool(name="wmps", bufs=1, space="PSUM") as wm_pool,
    ):
        # ---- DMA order is load-bearing: g00 feeds the warmup matmuls ----
        g_t = [[None] * NB for _ in range(GDEP)]
        g_t[0][0] = cpool.tile([128, N], BF16, name="g0_0")
        nc.sync.dma_start(g_t[0][0][:], G_d[0, 0:128, :])

        w_t = cpool.tile([128, C_OUT], BF16, name="w_t")
        nc.sync.dma_start(w_t[:], W_d[:])
        b_t = cpool.tile([128, 1], F32, name="b_t")
        nc.sync.dma_start(b_t[:], b_d[:])
        i128 = cpool.tile([128, 128], BF16, name="i128")
        nc.sync.dma_start(i128[:], I_d[:])

        # HAM warmup on the PE while the bulk DMAs land.
        wm = wm_pool.tile([128, N], F32, name="wm")
        for _ in range(cfg.n_warm):
            nc.tensor.matmul(
                wm[:], lhsT=g_t[0][0][:, 0:128], rhs=g_t[0][0][:],
                start=True, stop=True,
            )

        # X slices needed first, then the remaining G tiles, then the rest.
        xw = [[None] * n_xq for _ in range(NB)]

        def load_xq(lq):
            for wb in range(NB):
                t = cpool.tile([128, Lq * C_IN], BF16, name=f"xw_{wb}_{lq}")
                nc.sync.dma_start(
                    t.rearrange("w (l c) -> w l c", c=C_IN),
                    X_d[wb, :, lq * Lq:(lq + 1) * Lq, :],
                )
                xw[wb][lq] = t

        load_xq(0)
        for k in range(GDEP):
            for wb in range(NB):
                if g_t[k][wb] is None:
                    t = cpool.tile([128, N], BF16, name=f"g{k}_{wb}")
                    nc.sync.dma_start(t[:], G_d[k, wb * 128:(wb + 1) * 128, :])
                    g_t[k][wb] = t
        for lq in range(1, n_xq):
            load_xq(lq)

        def emit_conv(h0s2, lp):
            """Channel-mix conv for the l-pair (lp, lp+1) + bias."""
            cvp = cvps_pool.tile([128, 512], F32, name="cvp")
            # h0s2 free layout: (vh 2, l 2, v 256); rhs n=512 = (l, v) per vh
            nc.tensor.matmul(
                cvp[0:64, :], lhsT=w_t[:], rhs=h0s2[:, 0:512],
                start=True, stop=True, tile_position=(0, 0),
                skip_group_check=True,
            )
            nc.tensor.matmul(
                cvp[64:128, :], lhsT=w_t[:], rhs=h0s2[:, 512:1024],
                start=True, stop=True, tile_position=(0, 64),
                skip_group_check=True,
            )
            dst = out_sb[:, (lp % Lc) * 256:(lp % Lc + 2) * 256]
            if (lp // 2) % 2 == 0:
                nc.scalar.add(dst, cvp[:], b_t[:, 0:1])
            else:
                nc.vector.tensor_scalar_add(
                    out=dst, in0=cvp[:], scalar1=b_t[:, 0:1]
                )

        out_sb = out_pool.tile([128, 256 * Lc], BF16, name="out_sb")
        h0s2 = None
        prev = None  # deferred conv so PE never waits on the evac
        for l in range(cfg.L):
            h0p = h0ps_pool.tile([128, N], F32, name="h0p")
            for wb in range(NB):
                st = wb == 0
                sp = wb == NB - 1
                xl = xw[wb][l // Lq][:, (l % Lq) * C_IN:(l % Lq + 1) * C_IN]
                # X-transpose: identity matmul in column group 0, n=128
                # into this block's own column range (no accumulation).
                nc.tensor.matmul(
                    h0p[0:32, wb * 128:(wb + 1) * 128], lhsT=xl, rhs=i128[:],
                    start=True, stop=True,
                    tile_position=(0, 0), skip_group_check=True,
                )
                for k in range(GDEP):
                    j = k + 1
                    nc.tensor.matmul(
                        h0p[32 * j:32 * (j + 1), :], lhsT=xl,
                        rhs=g_t[k][wb][:],
                        start=st, stop=sp, tile_position=(0, 32 * j),
                        skip_group_check=True,
                    )
            if l % 2 == 0:
                h0s2 = h0sb_pool.tile([128, 1024], BF16, name="h0s2")
            # evac into v-half-major slots: (vh 2, l 2, v 256)
            dst = h0s2.rearrange("p (vh l v) -> p vh l v", vh=2, l=2)[
                :, :, l % 2, :
            ]
            if l % 2 == 0:
                nc.vector.tensor_copy(out=dst, in_=h0p[:])
            else:
                nc.scalar.copy(dst, h0p[:])

            if l % 2 == 1:
                if prev is not None:
                    emit_conv(*prev)
                    lp = prev[1]
                    if lp % Lc == Lc - 2:  # chunk complete -> store it
                        ch = lp // Lc
                        nc.sync.dma_start(
                            out_d[ch],
                            out_sb.rearrange("p (l v) -> p l v", v=256),
                        )
                        if ch + 1 < n_chunks:
                            out_sb = out_pool.tile(
                                [128, 256 * Lc], BF16, name="out_sb"
                            )
                prev = (h0s2, l - 1)
        emit_conv(*prev)
        nc.sync.dma_start(
            out_d[n_chunks - 1],
            out_sb.rearrange("p (l v) -> p l v", v=256),
        )


def build_nc(cfg: CFG):
    nc = bacc.Bacc("TRN2", target_bir_lowering=False, debug=False)
    n_chunks = cfg.L // cfg.Lc
    ins = {
        "xw": nc.dram_tensor("xw", [NB, 128, cfg.L, C_IN], BF16,
                             kind="ExternalInput").ap(),
        "gt": nc.dram_tensor("gt", [GDEP, N, N], BF16,
                             kind="ExternalInput").ap(),
        "ident": nc.dram_tensor("ident", [128, 128], BF16,
                                kind="ExternalInput").ap(),
        "wt": nc.dram_tensor("wt", [128, C_OUT], BF16,
                             kind="ExternalInput").ap(),
        "bias2": nc.dram_tensor("bias2", [128, 1], F32,
                                kind="ExternalInput").ap(),
    }
    outs = {
        "out": nc.dram_tensor("out", [n_chunks, 128, cfg.Lc, 256], BF16,
                              kind="ExternalOutput").ap(),
    }
    with tile.TileContext(nc) as tc:
        body(nc, tc, outs, ins, cfg)
    nc.compile()
    return nc


def make_host_inputs(X, A, W, b):
    """Precompute all device operands on the host."""
    import ml_dtypes
    bf16 = ml_dtypes.bfloat16

    A = np.asarray(A, np.float64)
    n = A.shape[0]
    An = A + np.eye(n)
    An = An / An.sum(axis=1, keepdims=True)
    As = (1.0 - BETA) * An
    eye = np.eye(n)
    G = []
    gk = eye
    for _ in range(GDEP):
        gk = As @ gk + BETA * eye
        G.append(gk)
    GT = np.stack([g.T for g in G]).astype(bf16)  # [GDEP, N, N]
    ident = np.eye(128, dtype=bf16)
    WT = np.ascontiguousarray(np.asarray(W, np.float64).T.astype(bf16))
    b = np.asarray(b, np.float32)
    b2 = np.ascontiguousarray(np.concatenate([b, b]).reshape(128, 1))

    # X [B, C_IN, N, L] f32 -> per core [NB, 128, L, C_IN] bf16 (lhsT layout)
    X = np.asarray(X)
    XW = np.ascontiguousarray(X.transpose(0, 2, 3, 1)).astype(bf16)
    XW = XW.reshape(B, NB, 128, L, C_IN)
    return XW, GT, ident, WT, b2


_NC_CACHE = {}


def run_on_hw(X, A, W, b, cfg=None, trace=False, **spmd_kwargs):
    XW, GT, ident, WT, b2 = make_host_inputs(X, A, W, b)
    if cfg is None:
        cfg = CFG()
    key = (cfg.L, cfg.Lc, cfg.Lq, cfg.n_warm)
    if key not in _NC_CACHE:
        _NC_CACHE[key] = build_nc(cfg)
    nc = _NC_CACHE[key]
    in_maps = [
        {"xw": XW[i], "gt": GT, "ident": ident, "wt": WT, "bias2": b2}
        for i in range(B)
    ]
    res = bass_utils.run_bass_kernel_spmd(
        nc, in_maps, core_ids=list(range(B)), trace=trace, **spmd_kwargs
    )
    # out_dev [n_chunks, 128=(vh,o), Lc, 256=v] bf16
    #   -> out [C_OUT, N, L] f32  via (o, vh, v, ch, l)
    n_chunks = cfg.L // cfg.Lc
    outs = []
    for i in range(B):
        o = np.asarray(res.results[i]["out"])
        o = o.reshape(n_chunks, 2, C_OUT, cfg.Lc, 256)
        o = o.transpose(2, 1, 4, 0, 3).reshape(C_OUT, N, cfg.L)
        outs.append(o.astype(np.float32))
    return np.stack(outs), res


def kernel(X, A, W, b):
    return run_on_hw(X, A, W, b)[0]


if __name__ == "__main__":
    rng = np.random.default_rng(0)
    X = rng.standard_normal((B, C_IN, N, L), dtype=np.float32)
    A = rng.random((N, N), dtype=np.float32)
    W = rng.standard_normal((C_OUT, (GDEP + 1) * C_IN), dtype=np.float32) * 0.1
    b = rng.random(C_OUT, dtype=np.float32)
    out = kernel(X, A, W, b)
    print("out", out.shape, out.dtype, float(np.abs(out).mean()))
